# revision 57
# baseline (speedup 1.0000x reference)
"""Trainium2 Bass kernel for nn_AttentionBlock (GroupNorm + single-head spatial
self-attention + residual) on 8 NeuronCores.

Sharding: data-parallel over batch (2) x sequence-parallel over the query
dimension (4 chunks of 1024 of the 4096 spatial tokens). Each core gets the
full image of its batch element, ROTATED so its query chunk sits at token 0
(GroupNorm stats, key/value sets and softmax sums are permutation-invariant
over tokens, so rotation lets all 8 cores run the identical SPMD program).

v3: fp8(e4m3) DoubleRow matmuls everywhere except the final wp projection
(bf16). Rel err ~6e-3 vs the 2e-2 gate (residual dominates the output and
softmax normalization self-corrects shared quantization error).

  - x is uploaded in two host-cast forms: the full image in the fp8
    DoubleRow pair layout (projection operand AND GroupNorm stats input;
    the fp8 quantization bias on mean/var is ~0.2%) and a bf16 copy of just
    the query quarter for the residual add. Zero on-device casts.
  - GroupNorm stats are engine-split: DVE bn_stats on 6 of 8 chunks per
    channel tile, ScalarE Copy/Square+accum_out on the rest, merged in a
    few column-vectorized DVE ops. The scale a then folds into a one-time
    bf16 -> fp8 a-scaled weight quantization (single rounding); the shift b
    becomes per-output-channel constants via tiny N=2 matmuls: q keeps qb,
    k's bias is DROPPED (a per-query additive logit constant is softmax-
    invariant), v's bias rides through softmax into yb.
  - throwaway matmul accumulation-groups, paced by the DMA/stats progress,
    keep the PE's HAM activity monitor at K=8/8 (2.4 GHz) through the
    engine-bound opening so real matmuls never run at the cold half clock.
  - projections (fp8 DR, contraction 256/pass): k/q weight-stationary, v
    x-stationary producing vT [token, channel] directly (no transposes).
  - attention per 256-key pair: scores^T = 2 DR MMs per 128-key tile, exp
    on ScalarE (shift -2.0 keeps exp < 448) straight to fp8 pairs, row-sums
    via a DR ones-matmul, AV accumulates over the 16 key pairs. The pair
    loop is software-pipelined (next pair's scores issue before this pair's
    rowsum/AV) so the PE streams through exp's ~700ns latency; measured
    99.5% PE busy at the 216 ns/MM fp8 streaming floor.
  - epilogue: 1/r via DVE reciprocal concurrent with the y matmuls (the
    normalization commutes with wp's channel mixing and is applied AFTER
    the projection), y = wpv@hattn*1/r + yb + x fused on DVE, outputs
    streamed per channel tile across all three DMA queues.
"""

import sys
from collections import deque
from contextlib import ExitStack

if "/opt/trn_rl_repo" not in sys.path:
    sys.path.insert(0, "/opt/trn_rl_repo")

import ml_dtypes
import numpy as np

import concourse.bass as bass  # noqa: F401  (import keeps bass registered)
import concourse.tile as tile
from concourse import bacc, mybir
from concourse.alu_op_type import AluOpType
from concourse.bass_utils import run_bass_kernel_spmd

F32 = mybir.dt.float32
F32R = mybir.dt.float32r
BF16 = mybir.dt.bfloat16
F8 = mybir.dt.float8e4
AF = mybir.ActivationFunctionType
OP = AluOpType
DR = mybir.MatmulPerfMode.DoubleRow

B, C, H, W = 2, 512, 64, 64
HW = H * W          # 4096 spatial tokens
P = 128             # partitions
CT = C // P         # 4 channel tiles
CP = CT // 2        # 2 channel-tile pairs (DoubleRow contraction groups)
NCORES = 8
QN = HW // 4        # 1024 queries per core
CHW = 512           # token chunk width
NCH = HW // CHW     # 8 chunks
JT = HW // P        # 32 key tiles
JP = JT // 2        # 16 key-tile pairs
EPS = 1e-6
SCALE = float(C) ** -0.5
SHIFT = 2.0         # exp(logit - SHIFT): keeps max exp ~190 < 448 (fp8 max)
GPT = P // 16       # 8 groups per channel tile


def _build_body(nc, tc, ctx, d):
    y_d = d["y"]

    cpool = ctx.enter_context(tc.tile_pool(name="const", bufs=1))
    ppool = ctx.enter_context(tc.tile_pool(name="persist", bufs=1))
    spool = ctx.enter_context(tc.tile_pool(name="stream", bufs=2))
    smpool = ctx.enter_context(tc.tile_pool(name="small", bufs=1))
    qpool = ctx.enter_context(tc.tile_pool(name="psum", bufs=3, space="PSUM"))

    dma_engines = [nc.gpsimd, nc.scalar, nc.sync]

    # ---- phase 1: stream fp8 x (host-cast); GroupNorm stats per chunk ----
    # x arrives in TWO host-prepared forms: the fp8 DoubleRow pair layout
    # xn8 (projection operand AND GroupNorm stats input - the quantization
    # bias on mean/var is ~0.2%, well inside tolerance) and a bf16 copy of
    # just the query quarter (residual add). No on-device casts at all.
    ind = cpool.tile([P, GPT], F32, tag="ind")
    nc.gpsimd.dma_start(ind[:], d["ind"][:])
    gps = qpool.tile([GPT, 2 * CT], F32, tag="pa")
    sts = [smpool.tile([P, NCH, 6], F32, tag="st", bufs=CT, name=f"st{t}")
           for t in range(CT)]
    xn8 = [ppool.tile([P, 2, HW], F8, tag=f"xn8_{p}", name=f"xn8_{p}")
           for p in range(CP)]
    # tile-major streaming: tile t completes before t+1, so the per-tile
    # stats chains below overlap the remaining tiles' DMA
    x8_engs = [nc.gpsimd, nc.sync, nc.gpsimd, nc.sync,
               nc.gpsimd, nc.sync, nc.gpsimd, nc.sync]
    for t in range(CT):
        for half in range(2):
            eng = x8_engs[t * 2 + half]
            hsl = slice(half * (HW // 2), (half + 1) * (HW // 2))
            eng.dma_start(xn8[t // 2][:, t % 2, hsl],
                          d["x8"][t // 2, :, t % 2, hsl])
    trash = qpool.tile([P, CHW], F32, tag="pr", bufs=1, name="trash")

    def _dummy_group(n, dep=None):
        # gap-free accumulation group of throwaway matmuls: keeps the PE HAM
        # at K=8/8 through engine-bound stretches where it would otherwise
        # idle past the MID window and re-throttle to half clock
        xd = xn8[0][:, 0, :]
        for r in range(n):
            nc.tensor.matmul(trash[:], xd[:, r * P:(r + 1) * P]
                             if dep is None else dep,
                             xd[:, 0:CHW], start=(r == 0), stop=(r == n - 1))
    # stats are split across engines: DVE runs bn_stats on chunks 0-5, SE
    # computes raw sum / sum-of-squares for chunks 6-7 in one 1024-wide
    # Copy/Square pass each (accum_out); the two are merged below in a few
    # column-vectorized ops so DVE's serial stats chain shrinks by ~25%
    NBN = 6
    mvall = smpool.tile([P, CT, 2], F32, tag="mvall")
    sxt = smpool.tile([P, CT, 2], F32, tag="sxt")
    junk = smpool.tile([P, (NCH - NBN) * CHW], F8, tag="junk", bufs=2)
    for t in range(CT):
        xt = xn8[t // 2][:, t % 2, :]
        for ch in range(NCH):
            sl = slice(ch * CHW, (ch + 1) * CHW)
            if ch < NBN:
                nc.vector.bn_stats(sts[t][:, ch, :], xt[:, sl])
            # dummy matmuls on the freshly-landed chunks keep the PE HAM warm
            # through the DMA-bound stats phase: one dense ~7us burst on
            # chunk 0 (an accumulation group: no WAW waits between members,
            # so the PE streams it gap-free and a full HAM SHORT window reads
            # busy -> K=8/8), then one matmul every other chunk defeats the
            # MID-idle re-throttle; results are discarded
            if (t, ch) == (0, 0):
                for r in range(16):
                    nc.tensor.matmul(trash[:], xt[:, ch * CHW:ch * CHW + P],
                                     xt[:, sl], start=(r == 0),
                                     stop=(r == 15))
            elif (ch + 1) % 2:
                nc.tensor.matmul(trash[:], xt[:, ch * CHW:ch * CHW + P],
                                 xt[:, sl], start=True, stop=True)
        tsl = slice(NBN * CHW, NCH * CHW)
        nc.scalar.activation(junk[:], xt[:, tsl], AF.Copy,
                             accum_out=sxt[:, t, 0:1])
        nc.scalar.activation(junk[:], xt[:, tsl], AF.Square,
                             accum_out=sxt[:, t, 1:2])
        nc.vector.bn_aggr(mvall[:, t, :], sts[t][:, 0:NBN, :])
        if t >= 1:
            # HAM maintenance through the stats tail: dummy groups paced by
            # the aggregation chain's progress
            _dummy_group(6)
    # merge: s2[:, t, 0] = mean, s2[:, t, 1] = E[x^2], all tiles at once
    na, nb, nn = float(NBN * CHW), float((NCH - NBN) * CHW), float(NCH * CHW)
    s2a = smpool.tile([P, CT, 2], F32, tag="s2a")
    nc.vector.tensor_tensor(s2a[:, :, 0], mvall[:, :, 0], mvall[:, :, 0],
                            op=OP.mult)
    nc.vector.tensor_tensor(s2a[:, :, 1], mvall[:, :, 1], s2a[:, :, 0],
                            op=OP.add)
    nc.vector.tensor_copy(s2a[:, :, 0], mvall[:, :, 0])
    s2b = smpool.tile([P, CT, 2], F32, tag="s2b")
    nc.vector.tensor_scalar(s2b[:], s2a[:], na / nn, None, OP.mult)
    s2f = smpool.tile([P, CT, 2], F32, tag="s2f")
    nc.vector.scalar_tensor_tensor(s2f[:], sxt[:], 1.0 / nn, s2b[:],
                                   op0=OP.mult, op1=OP.add)
    _dummy_group(6)
    for t in range(CT):
        nc.tensor.matmul(gps[:, 2 * t:2 * t + 2], ind[:], s2f[:, t, :],
                         start=True, stop=True)

    # ---- small constants (after the x stream in every trigger queue) ----
    chvv = cpool.tile([P, CT, 6], F32, tag="chvv")
    nc.gpsimd.dma_start(chvv[:], d["chv"][:])
    # chvv columns: gamma, beta, bq, bk, bv, bp
    indT = cpool.tile([GPT, P], F32, tag="indT")
    nc.gpsimd.dma_start(indT[:], d["indT"][:])
    # f32r projection weights (transposed [c, o]): consumed by the tiny bias
    # contracts and by the one-time a-scaled fp8 quantization below
    wts = {}
    for wi, name in enumerate(("wkT", "wvT", "wqT")):
        w = cpool.tile([P, CT, C], BF16, tag=name, name=name)
        [nc.gpsimd, nc.sync, nc.gpsimd][wi].dma_start(w[:], d[name][:])
        wts[name] = [w[:, t, :] for t in range(CT)]
    # bf16 residual x (query quarter only), needed first at the half-0 tail
    xqv = cpool.tile([P, CT, QN], BF16, tag="xqv", name="xqv")
    nc.sync.dma_start(xqv[:], d["xq"][:])
    xq = [xqv[:, t, :] for t in range(CT)]
    # fp8 ones for the DoubleRow row-sum matmul: M=16 columns because the
    # dual-fp8 LDWEIGHTS ISA check requires the pair step to be 16-aligned
    # (and fp8 memset is not a legal ISA instruction -> host constant)
    ones2 = cpool.tile([P, 2, 16], F8, tag="ones2")
    nc.gpsimd.dma_start(ones2[:], d["ones8"][:])
    ones_r32 = smpool.tile([1, P], F32R, tag="onesr32")
    nc.gpsimd.dma_start(ones_r32[:], d["onesr"][:])
    epst = smpool.tile([GPT, 1], F32, tag="eps")
    nc.vector.memset(epst[:], EPS)
    shiftt = smpool.tile([P, 1], F32, tag="shift")
    nc.vector.memset(shiftt[:], -SHIFT)

    gst = smpool.tile([GPT, 2 * CT], F32, tag="gst")
    nc.vector.tensor_copy(gst[:], gps[:])
    g3 = gst.rearrange("p (t two) -> p t two", two=2)
    msq = smpool.tile([GPT, CT], F32, tag="msq")
    nc.vector.tensor_tensor(msq[:], g3[:, :, 0], g3[:, :, 0], op=OP.mult)
    varg = smpool.tile([GPT, CT], F32, tag="varg")
    nc.vector.tensor_tensor(varg[:], g3[:, :, 1], msq[:], op=OP.subtract)
    stdg = smpool.tile([GPT, CT], F32, tag="stdg")
    nc.scalar.activation(stdg[:], varg[:], AF.Sqrt, bias=epst[:])
    # interleave (mu_t, rstd_t) columns and broadcast all groups->channels
    # with a single [K=8, M=128, N=8] indicator matmul
    mr = smpool.tile([GPT, 2 * CT], F32, tag="mr")
    mr3 = mr.rearrange("p (t two) -> p t two", two=2)
    nc.vector.tensor_copy(mr3[:, :, 0], g3[:, :, 0])
    nc.vector.reciprocal(mr3[:, :, 1], stdg[:])
    cba = qpool.tile([P, 2 * CT], F32, tag="pa")
    nc.tensor.matmul(cba[:], indT[:], mr[:], start=True, stop=True)
    cb = smpool.tile([P, 2 * CT], F32, tag="cb")
    nc.vector.tensor_copy(cb[:], cba[:])

    # per-channel Scale a / Bias b, all tiles in one column-vectorized pass
    cb3 = cb.rearrange("p (t two) -> p t two", two=2)
    aall = ppool.tile([P, CT], F32, tag="aall", name="aall")
    nc.vector.tensor_tensor(aall[:], cb3[:, :, 1], chvv[:, :, 0], op=OP.mult)
    tmpb = smpool.tile([P, CT], F32, tag="tmpb", bufs=1)
    nc.vector.tensor_tensor(tmpb[:], cb3[:, :, 0], aall[:], op=OP.mult)
    ball = smpool.tile([P, CT], F32, tag="ball")
    nc.vector.tensor_tensor(ball[:], chvv[:, :, 1], tmpb[:], op=OP.subtract)
    bvec = ppool.tile([P, CT, 2], BF16, tag="bvec", name="bvec")
    nc.vector.tensor_copy(bvec[:, :, 0], ball[:])
    nc.vector.tensor_copy(bvec[:, :, 1], ball[:])
    _dummy_group(5)

    # ---- one-time a-scaled fp8 weight quantization (single rounding) ----
    # w8s[name][p][cp, t, o] = fp8( wT[(2p+t)*128+cp, o] * a[(2p+t)*128+cp] )
    w8s = {}
    for name in ("wkT", "wvT", "wqT"):
        w8s[name] = [cpool.tile([P, 2, C], F8, tag=f"{name}8_{p}",
                                name=f"{name}8_{p}") for p in range(CP)]
        for t in range(CT):
            dst = w8s[name][t // 2][:, t % 2, :]
            if t % 2 == 0:
                nc.vector.tensor_scalar_mul(dst, wts[name][t],
                                            aall[:, t:t + 1])
            else:
                nc.scalar.activation(dst, wts[name][t], AF.Copy,
                                     scale=aall[:, t:t + 1])
        if name == "wkT":
            # bridge matmuls: depend on exactly what phase 2's first real
            # matmul needs, so the PE stays HAM-warm through the stats tail
            for p in range(CP):
                nc.tensor.matmul(trash[:], w8s[name][p][:, :, 0:P],
                                 xn8[p][:, :, 0:CHW], start=True, stop=True,
                                 perf_mode=DR)

    # ---- bias-term constants from UNSCALED weights (tiny N=2 matmuls) ----
    #   qb[o] = sum_c wq[o,c] b[c] + bq    (per-partition add at the q copy)
    #   vbt[c] = sum_cin wv[c,cin] b[cin] + bv   (rides softmax into yb)
    #   yb[o] = sum_c wp[o,c] vbt[c] + bp        (y epilogue constant)
    #   (k needs NO bias: a per-query logit constant is softmax-invariant)
    def bias_contract(wtiles, rhs_tiles, outdt, addcol, tagp, two_col=False):
        outs = []
        for ot in range(CT):
            pb = qpool.tile([P, 2], F32, tag="pa")
            for t in range(CT):
                nc.tensor.matmul(pb[:], wtiles[t][:, ot * P:(ot + 1) * P],
                                 rhs_tiles[t], start=(t == 0),
                                 stop=(t == CT - 1))
            w = 2 if two_col else 1
            ob = ppool.tile([P, w], outdt, tag=f"{tagp}{ot}", name=f"{tagp}{ot}")
            nc.vector.tensor_scalar(ob[:], pb[:, 0:w],
                                    chvv[:, ot, addcol:addcol + 1],
                                    None, OP.add)
            outs.append(ob)
        return outs

    _dummy_group(5)
    vbt = bias_contract(wts["wvT"],
                        [bvec[:, t, :] for t in range(CT)],
                        BF16, 4, "vbt", two_col=True)
    qb = bias_contract(wts["wqT"],
                       [bvec[:, t, :] for t in range(CT)], F32, 2, "qb")

    # ---- persistent attention operands (all fp8, DoubleRow layouts) ----
    # k2[p]  : [128, j-tile, pair-slot, 128]   stationary slices [:, j, :, :]
    # q2[p]  : [128, pair-slot, 1024]          moving slices [:, :, i-half]
    # xn8[p] : [128, pair-slot, 4096]          moving (k/q) + stationary (v)
    # vT2[jp]: [128, c-tile, pair-slot, 128]   stationary slices [:, t, :, :]
    k2 = [ppool.tile([P, JT, 2, P], F8, tag=f"k2_{p}", name=f"k2_{p}")
          for p in range(CP)]
    q2 = [ppool.tile([P, 2, QN], F8, tag=f"q2_{p}", name=f"q2_{p}")
          for p in range(CP)]
    vT2 = [ppool.tile([P, CT, 2, P], F8, tag=f"vT2_{jp}", name=f"vT2_{jp}")
           for jp in range(JP)]

    # ---- phase 2: q/k/v fp8 projections, streamed over x chunks ----
    for ch in range(NCH):
        sl = slice(ch * CHW, (ch + 1) * CHW)
        for ot in range(CT):
            pk = qpool.tile([P, CHW], F32, tag="pa")
            for p in range(CP):
                nc.tensor.matmul(pk[:],
                                 w8s["wkT"][p][:, :, ot * P:(ot + 1) * P],
                                 xn8[p][:, :, sl], start=(p == 0),
                                 stop=(p == CP - 1), perf_mode=DR)
            # k write: [128, 4 j-tiles, 1, 128] strided into the pair layout
            dst = k2[ot // 2][:, 4 * ch:4 * ch + 4, ot % 2, :]
            if ot % 2 == 0:
                nc.vector.tensor_copy(dst, pk[:])
            else:
                nc.scalar.copy(dst, pk[:])
            del dst
        for tg in range(CT):
            jt = ch * CT + tg
            pv = qpool.tile([P, CHW], F32, tag="pa")
            for p in range(CP):
                nc.tensor.matmul(
                    pv[:], xn8[p][:, :, jt * P:(jt + 1) * P],
                    w8s["wvT"][p][:], start=(p == 0), stop=(p == CP - 1),
                    perf_mode=DR)
            dst = vT2[jt // 2][:, :, jt % 2, :]
            if tg % 2 == 1:
                nc.scalar.copy(dst, pv[:])
            else:
                nc.vector.tensor_copy(dst, pv[:])
            del dst
        if ch * CHW < QN:
            for ot in range(CT):
                pq = qpool.tile([P, CHW], F32, tag="pa")
                for p in range(CP):
                    nc.tensor.matmul(
                        pq[:], w8s["wqT"][p][:, :, ot * P:(ot + 1) * P],
                        xn8[p][:, :, sl], start=(p == 0), stop=(p == CP - 1),
                        perf_mode=DR)
                if ot % 2 == 0:
                    nc.scalar.add(q2[ot // 2][:, ot % 2, sl], pq[:],
                                  qb[ot][:, 0:1])
                else:
                    nc.vector.tensor_scalar(q2[ot // 2][:, ot % 2, sl], pq[:],
                                            qb[ot][:], None, OP.add)

    # ---- phase 3: attention, per query half ----
    # wpT (f32r) loads late: only the y epilogue needs it
    wpTv = cpool.tile([P, CT, C], BF16, tag="wpT", name="wpT")
    nc.sync.dma_start(wpTv[:], d["wpT"][:])
    wpT = [wpTv[:, t, :] for t in range(CT)]
    yb = []
    for ot in range(CT):
        pb = qpool.tile([P, 2], F32, tag="pa")
        for t in range(CT):
            nc.tensor.matmul(pb[:], wpT[t][:, ot * P:(ot + 1) * P],
                             vbt[t][:, 0:2], start=(t == 0), stop=(t == CT - 1))
        ob = ppool.tile([P, 1], F32, tag=f"yb{ot}", name=f"yb{ot}")
        nc.vector.tensor_scalar(ob[:], pb[:, 0:1], chvv[:, ot, 5:6],
                                None, OP.add)
        yb.append(ob)

    def mk_pr():
        return qpool.tile([16, CHW], F32, tag="pr", bufs=1, name="pr")

    def mk_po():
        return [qpool.tile([P, CHW], F32, tag=f"po{t}", name=f"po{t}", bufs=1)
                for t in range(CT)]

    def score_pair(ih, jp):
        """scores^T + exp for key tiles (2jp, 2jp+1) -> one fp8 pT2 pair."""
        isl = slice(ih * CHW, (ih + 1) * CHW)
        pT2 = spool.tile([P, 2, CHW], F8, tag="pT2", bufs=8, name="pT2")
        for jj in range(2):
            j = 2 * jp + jj
            ps_ = qpool.tile([P, CHW], F32, tag="pa", name="ps")
            for p in range(CP):
                nc.tensor.matmul(ps_[:], k2[p][:, j, :, :], q2[p][:, :, isl],
                                 start=(p == 0), stop=(p == CP - 1),
                                 perf_mode=DR)
            nc.scalar.activation(pT2[:, jj, :], ps_[:], AF.Exp,
                                 scale=SCALE, bias=shiftt[:])
        return pT2

    def av_pair(pr, po, jp, pT2):
        nc.tensor.matmul(pr[:], ones2[:], pT2[:], start=(jp == 0),
                         stop=(jp == JP - 1), perf_mode=DR)
        for t in range(CT):
            nc.tensor.matmul(po[t][:], vT2[jp][:, t, :, :], pT2[:],
                             start=(jp == 0), stop=(jp == JP - 1),
                             perf_mode=DR)

    def tail_and_y(pr, po, ih, nsub=1):
        # nsub>1 splits the epilogue into query sub-slices so the final
        # drain pipelines DVE normalize / PE matmul / DMA out
        sw = CHW // nsub
        for sub in range(nsub):
            lo = ih * CHW + sub * sw
            isl = slice(lo, lo + sw)
            psl = slice(sub * sw, (sub + 1) * sw)
            # softmax normalization applied AFTER the wp projection (1/r
            # is per-query-column, so it commutes with wp's channel mixing):
            # the 1/r chain runs concurrently with the y matmuls instead of
            # gating them
            rsb = spool.tile([1, sw], F32R, tag="sx", bufs=3)
            with nc.allow_low_precision(reason="f32r 1/r, bits == f32"):
                nc.vector.reciprocal(rsb[:], pr[0:1, psl])
            prb = qpool.tile([P, sw], F32, tag="pa")
            nc.tensor.matmul(prb[:], ones_r32[:], rsb[:], start=True, stop=True)
            rb = spool.tile([P, sw], F32, tag="sx", bufs=3)
            nc.vector.tensor_copy(rb[:], prb[:])
            has = []
            for t in range(CT):
                ha = spool.tile([P, sw], BF16, tag=f"hx{t}", bufs=2)
                if t % 2 == 0:
                    nc.scalar.copy(ha[:], po[t][:, psl])
                else:
                    nc.vector.tensor_copy(ha[:], po[t][:, psl])
                has.append(ha)
            for ot in range(CT):
                py = qpool.tile([P, sw], F32, tag="pa")
                for t in range(CT):
                    nc.tensor.matmul(py[:], wpT[t][:, ot * P:(ot + 1) * P],
                                     has[t][:], start=(t == 0),
                                     stop=(t == CT - 1))
                pyn = spool.tile([P, sw], F32, tag=f"hx{ot}", bufs=2)
                nc.vector.tensor_tensor(pyn[:], py[:], rb[:], op=OP.mult)
                yt = spool.tile([P, sw], F32, tag="yt", bufs=4, name="yt")
                nc.vector.scalar_tensor_tensor(yt[:], pyn[:], yb[ot][:, 0:1],
                                               xq[ot][:, isl],
                                               op0=OP.add, op1=OP.add)
                eng = [nc.gpsimd, nc.sync, nc.scalar][(ot + sub) % 3]
                eng.dma_start(y_d[ot, :, isl], yt[:])

    # software pipeline: pair jp+1's score matmuls are emitted BEFORE pair
    # jp's rowsum/AV so the PE streams through exp's ~700ns latency instead
    # of stalling on it; KPRE extra half-1 pairs cover half-0's epilogue
    KPRE = 6
    sq_ = deque()
    pr0 = mk_pr()
    po0 = mk_po()
    sq_.append(score_pair(0, 0))
    for jp in range(JP):
        if jp + 1 < JP:
            sq_.append(score_pair(0, jp + 1))
        else:
            sq_.append(score_pair(1, 0))
        av_pair(pr0, po0, jp, sq_.popleft())
    pr1 = mk_pr()
    for jp in range(1, KPRE):
        sq_.append(score_pair(1, jp))
    tail_and_y(pr0, po0, 0)
    po1 = mk_po()
    for jp in range(JP):
        if jp + KPRE < JP:
            sq_.append(score_pair(1, jp + KPRE))
        av_pair(pr1, po1, jp, sq_.popleft())
    tail_and_y(pr1, po1, 1)


def build_module():
    nc = bacc.Bacc("TRN2", target_bir_lowering=False, debug=False,
                   num_devices=NCORES)
    d = {
        "x8": nc.dram_tensor("x8", [CP, P, 2, HW], F8,
                             kind="ExternalInput").ap(),
        "xq": nc.dram_tensor("xq", [P, CT, QN], BF16,
                             kind="ExternalInput").ap(),
        "wqT": nc.dram_tensor("wqT", [P, CT, C], BF16,
                              kind="ExternalInput").ap(),
        "wkT": nc.dram_tensor("wkT", [P, CT, C], BF16,
                              kind="ExternalInput").ap(),
        "wvT": nc.dram_tensor("wvT", [P, CT, C], BF16,
                              kind="ExternalInput").ap(),
        "wpT": nc.dram_tensor("wpT", [P, CT, C], BF16,
                              kind="ExternalInput").ap(),
        "chv": nc.dram_tensor("chv", [P, CT, 6], F32,
                              kind="ExternalInput").ap(),
        "ones8": nc.dram_tensor("ones8", [P, 2, 16], F8,
                                kind="ExternalInput").ap(),
        "onesr": nc.dram_tensor("onesr", [1, P], F32R,
                                kind="ExternalInput").ap(),
        "ind": nc.dram_tensor("ind", [P, GPT], F32, kind="ExternalInput").ap(),
        "indT": nc.dram_tensor("indT", [GPT, P], F32, kind="ExternalInput").ap(),
        "y": nc.dram_tensor("y", [CT, P, QN], F32, kind="ExternalOutput").ap(),
    }
    with tile.TileContext(nc) as tc, ExitStack() as ctx:
        _build_body(nc, tc, ctx, d)
    nc.compile()
    return nc


_CACHE = {}


def _get_nc():
    if "nc" not in _CACHE:
        _CACHE["nc"] = build_module()
    return _CACHE["nc"]


def _shared_inputs(gamma, beta, wq, bq, wk, bk, wv, bv, wp, bp):
    def wT(w):
        return np.ascontiguousarray(
            np.asarray(w, np.float32).T.reshape(CT, P, C)
            .transpose(1, 0, 2)).astype(ml_dtypes.bfloat16)

    ind = np.zeros((P, GPT), np.float32)
    for i in range(P):
        ind[i, i // 16] = 1.0 / 16.0
    indT = np.zeros((GPT, P), np.float32)
    for i in range(P):
        indT[i // 16, i] = 1.0
    chv = np.stack([np.asarray(a, np.float32)
                    for a in (gamma, beta, bq, bk, bv, bp)],
                   axis=1).reshape(CT, P, 6).transpose(1, 0, 2)
    return {
        "wqT": wT(wq), "wkT": wT(wk), "wvT": wT(wv),
        "wpT": wT(wp),
        "chv": np.ascontiguousarray(chv),
        "ones8": np.ones((P, 2, 16), np.float32).astype(ml_dtypes.float8_e4m3fn),
        "onesr": np.ones((1, P), np.float32),
        "ind": ind, "indT": indT,
    }


def make_in_maps(x, gamma, beta, wq, bq, wk, bk, wv, bv, wp, bp):
    shared = _shared_inputs(gamma, beta, wq, bq, wk, bk, wv, bv, wp, bp)
    xf = np.asarray(x, np.float32).reshape(B, C, HW)
    in_maps = []
    for core in range(NCORES):
        b, qc = divmod(core, NCORES // B)
        xb = np.roll(xf[b], -qc * QN, axis=1)          # [C, HW]
        x8 = xb.reshape(CP, 2, P, HW).transpose(0, 2, 1, 3)
        m = dict(shared)
        m["x8"] = np.ascontiguousarray(x8).astype(ml_dtypes.float8_e4m3fn)
        m["xq"] = np.ascontiguousarray(
            xb.reshape(CT, P, HW)[:, :, :QN].transpose(1, 0, 2)
        ).astype(ml_dtypes.bfloat16)
        in_maps.append(m)
    return in_maps


def assemble_output(results):
    out = np.empty((B, C, HW), np.float32)
    for core in range(NCORES):
        b, qc = divmod(core, NCORES // B)
        y = np.asarray(results[core]["y"]).reshape(C, QN)
        out[b, :, qc * QN:(qc + 1) * QN] = y
    return out.reshape(B, C, H, W)


def kernel(x, gamma, beta, wq, bq, wk, bk, wv, bv, wp, bp):
    nc = _get_nc()
    in_maps = make_in_maps(x, gamma, beta, wq, bq, wk, bk, wv, bv, wp, bp)
    res = run_bass_kernel_spmd(nc, in_maps, list(range(NCORES)))
    return assemble_output(res.results)
